# revision 14
# baseline (speedup 1.0000x reference)
"""Trainium2 Bass kernel: MACE TensorProductConv (scalar-in x SH tensor product
+ per-edge MLP weights + segment_sum onto destination nodes).

Strategy (dst-sharded, no collectives):
  - Host bin-packs the N_DST destination nodes into 80 bins (<=128 nodes each,
    edge-balanced). Core k owns 10 bins ("chunks"); each chunk's node outputs
    accumulate in PSUM [128 nodes, 2048].
  - Edges are grouped by bin and padded to BLOCKS*128 slots per bin. Per
    128-edge block the device computes the edge MLP (2 matmuls + silu), the
    per-edge tensor product (DVE broadcast multiplies), and scatter-adds via a
    one-hot matmul: PSUM += onehot(dst_local)^T @ out_e.
  - Host pre-gathers src_features[src], pre-transposes edge_emb, pre-scales
    the MLP weights, and un-permutes the device output at the end.
"""

import sys
from contextlib import ExitStack

for _p in ("/opt/trn_rl_repo",):
    if _p not in sys.path:
        sys.path.insert(0, _p)

import numpy as np

import concourse.bacc as bacc
import concourse.tile as tile
import concourse.mybir as mybir
from concourse.bass_utils import run_bass_kernel_spmd

F32 = mybir.dt.float32
F32R = mybir.dt.float32r
BF16 = mybir.dt.bfloat16

N_CORES = 8
P = 128                 # partitions = edges per block = nodes per chunk
CHUNKS = 10             # bins per core
NBINS = N_CORES * CHUNKS
C = 128                 # channels
SH_DIM = 16
MLP_IN = 64
MLP_HID = 128
W_NUMEL = 512
OUT_COLS = 16 * C       # 2048
D_L = (1, 3, 5, 7)
OFF_L = (0, 1, 4, 9)    # m-offset of each l within the 16 sh columns
PAD_DST = 1000.0        # dst_local for padding slots -> one-hot row is all zero


# ---------------------------------------------------------------------------
# device program
# ---------------------------------------------------------------------------

_NC_CACHE = {}


def _build(blocks):
    nc = bacc.Bacc("TRN2", target_bir_lowering=False, debug=False,
                   num_devices=N_CORES)
    S = blocks * P
    embt = nc.dram_tensor("embt", [CHUNKS, MLP_IN, S], F32, kind="ExternalInput").ap()
    shp = nc.dram_tensor("shp", [CHUNKS, blocks, P, SH_DIM], F32, kind="ExternalInput").ap()
    xp = nc.dram_tensor("xp", [CHUNKS, blocks, P, C], F32, kind="ExternalInput").ap()
    dstf = nc.dram_tensor("dstf", [CHUNKS, P, blocks], F32, kind="ExternalInput").ap()
    w1s = nc.dram_tensor("w1s", [MLP_IN, MLP_HID], F32, kind="ExternalInput").ap()
    w2s = nc.dram_tensor("w2s", [MLP_HID, W_NUMEL], F32, kind="ExternalInput").ap()
    iota = nc.dram_tensor("iota", [P, P], F32, kind="ExternalInput").ap()
    outp = nc.dram_tensor("out", [CHUNKS, P, OUT_COLS], F32, kind="ExternalOutput").ap()

    with tile.TileContext(nc) as tc, ExitStack() as ctx:
        const = ctx.enter_context(tc.tile_pool(name="const", bufs=1))
        cpool = ctx.enter_context(tc.tile_pool(name="chunk", bufs=2))
        bpool = ctx.enter_context(tc.tile_pool(name="blk", bufs=3))
        opool = ctx.enter_context(tc.tile_pool(name="outs", bufs=2))
        ph_pool = ctx.enter_context(tc.tile_pool(name="ph", bufs=2, space="PSUM"))
        pw_pool = ctx.enter_context(tc.tile_pool(name="pw", bufs=2, space="PSUM"))
        pacc_pool = ctx.enter_context(tc.tile_pool(name="pacc", bufs=1, space="PSUM"))

        w1t = const.tile([MLP_IN, MLP_HID], F32)
        nc.sync.dma_start(w1t[:], w1s)
        w2t = const.tile([MLP_HID, W_NUMEL], F32)
        nc.sync.dma_start(w2t[:], w2s)
        iot = const.tile([P, P], F32)
        nc.sync.dma_start(iot[:], iota)

        for ch in range(CHUNKS):
            et = cpool.tile([MLP_IN, S], F32, tag="et")
            nc.sync.dma_start(et[:], embt[ch])
            xt = cpool.tile([P, S], F32, tag="xt")
            nc.sync.dma_start(xt[:].rearrange("p (b c) -> p b c", b=blocks),
                              xp[ch].rearrange("b i c -> i b c"))
            sht = cpool.tile([P, blocks * SH_DIM], BF16, tag="sht")
            nc.gpsimd.dma_start(sht[:].rearrange("p (b k) -> p b k", b=blocks),
                                shp[ch].rearrange("b i k -> i b k"))
            shtf = cpool.tile([P, blocks * SH_DIM], F32, tag="shtf")
            nc.sync.dma_start(shtf[:].rearrange("p (b k) -> p b k", b=blocks),
                              shp[ch].rearrange("b i k -> i b k"))
            dtt = cpool.tile([P, blocks], F32, tag="dtt")
            nc.sync.dma_start(dtt[:], dstf[ch])

            acc = pacc_pool.tile([P, OUT_COLS], F32)

            for blk in range(blocks):
                # --- edge MLP: hT = silu(W1s^T @ emb^T), w = (hT)^T @ W2s ---
                # float32r: fp32 data, 4x faster PE streaming at N>=256
                ph = ph_pool.tile([MLP_HID, P], F32)
                nc.tensor.matmul(ph[:], lhsT=w1t[:],
                                 rhs=et[:, blk * P:(blk + 1) * P],
                                 start=True, stop=True)
                hs = bpool.tile([MLP_HID, P], F32, tag="hs")
                nc.scalar.activation(hs[:], ph[:], mybir.ActivationFunctionType.Silu)
                pw = pw_pool.tile([P, W_NUMEL], F32)
                nc.tensor.matmul(pw[:], lhsT=hs[:], rhs=w2t[:],
                                 start=True, stop=True)

                # --- A[e, l*C+c] = w[e, l*C+c] * x[e, c] ---
                xb = xt[:, blk * P:(blk + 1) * P]
                at = bpool.tile([P, W_NUMEL], BF16, tag="at")
                nc.vector.tensor_mul(
                    at[:].rearrange("p (l c) -> p l c", l=4),
                    pw[:].rearrange("p (l c) -> p l c", l=4),
                    xb.unsqueeze(1).broadcast_to([P, 4, C]),
                )

                # --- one-hot of dst_local (bf16: 0/1 exact) ---
                oh = bpool.tile([P, P], BF16, tag="oh")
                nc.vector.tensor_scalar(oh[:], iot[:], dtt[:, blk:blk + 1], None,
                                        mybir.AluOpType.is_equal)

                # --- out_e[e, (off+m)*C + c] = A[e, l*C+c] * sh[e, off+m] ---
                # split across engines: l=0,2 -> DVE, l=1 -> ACT (copy w/
                # per-partition scale), l=3 -> GPSIMD
                oe = bpool.tile([P, OUT_COLS], BF16, tag="oe")
                for li, eng in ((0, nc.vector), (2, nc.vector), (3, nc.gpsimd)):
                    d, off = D_L[li], OFF_L[li]
                    eng.tensor_mul(
                        oe[:, off * C:(off + d) * C].rearrange("p (d c) -> p d c", d=d),
                        at[:, li * C:(li + 1) * C].unsqueeze(1).broadcast_to([P, d, C]),
                        sht[:, blk * SH_DIM + off: blk * SH_DIM + off + d]
                            .unsqueeze(2).broadcast_to([P, d, C]),
                    )
                for m in range(D_L[1]):
                    off = OFF_L[1]
                    nc.scalar.activation(
                        oe[:, (off + m) * C:(off + m + 1) * C],
                        at[:, C:2 * C],
                        mybir.ActivationFunctionType.Copy,
                        scale=shtf[:, blk * SH_DIM + off + m: blk * SH_DIM + off + m + 1],
                    )

                # --- scatter-add: acc += onehot^T @ out_e ---
                for j in range(4):
                    nc.tensor.matmul(acc[:, j * 512:(j + 1) * 512],
                                     lhsT=oh[:], rhs=oe[:, j * 512:(j + 1) * 512],
                                     start=(blk == 0), stop=(blk == blocks - 1))

            ot = opool.tile([P, OUT_COLS], F32, tag="ot")
            nc.scalar.copy(ot[:], acc[:])
            nc.sync.dma_start(outp[ch], ot[:])

    nc.compile()
    return nc


def _get_nc(blocks):
    if blocks not in _NC_CACHE:
        _NC_CACHE[blocks] = _build(blocks)
    return _NC_CACHE[blocks]


# ---------------------------------------------------------------------------
# host-side packing
# ---------------------------------------------------------------------------

def _bin_pack(dst, n_dst):
    """Assign each dst node to one of NBINS bins: <=P nodes per bin,
    edge counts balanced (greedy LPT)."""
    import heapq
    deg = np.bincount(dst, minlength=n_dst)
    order = np.argsort(-deg, kind="stable")
    heap = [(0, b) for b in range(NBINS)]
    heapq.heapify(heap)
    bin_of_node = np.empty(n_dst, np.int32)
    bin_nnodes = np.zeros(NBINS, np.int32)
    bin_nedges = np.zeros(NBINS, np.int64)
    for n in order:
        while True:
            e, b = heapq.heappop(heap)
            if bin_nnodes[b] < P:
                break
        bin_of_node[n] = b
        bin_nnodes[b] += 1
        bin_nedges[b] = e + deg[n]
        if bin_nnodes[b] < P:
            heapq.heappush(heap, (int(bin_nedges[b]), b))
    return bin_of_node, bin_nnodes, bin_nedges


def _pack(src_features, edge_sh, edge_emb, W1, W2, src, dst, n_dst):
    E = dst.shape[0]
    bin_of_node, bin_nnodes, bin_nedges = _bin_pack(dst, n_dst)
    blocks = max(1, int(-(-int(bin_nedges.max()) // P)))  # ceil
    S = blocks * P

    # local node index within bin
    norder = np.argsort(bin_of_node, kind="stable")
    nstarts = np.concatenate([[0], np.cumsum(bin_nnodes)[:-1]])
    local_idx = np.empty(n_dst, np.int64)
    local_idx[norder] = np.arange(n_dst) - nstarts[bin_of_node[norder]]
    node_slot = bin_of_node.astype(np.int64) * P + local_idx

    # group edges by bin -> slot table [NBINS, S] of edge ids (-1 = padding)
    ebin = bin_of_node[dst]
    eorder = np.argsort(ebin, kind="stable")
    counts = np.bincount(ebin, minlength=NBINS)
    estarts = np.concatenate([[0], np.cumsum(counts)[:-1]])
    pos_sorted = np.arange(E) - estarts[ebin[eorder]]
    slot_edge = np.full((NBINS, S), -1, np.int64)
    slot_edge[ebin[eorder], pos_sorted] = eorder

    # gather per-slot data (index -1 -> appended zero row)
    ee = np.concatenate([edge_emb, np.zeros((1, MLP_IN), np.float32)])
    es = np.concatenate([edge_sh, np.zeros((1, SH_DIM), np.float32)])
    xs = np.concatenate([src_features[src], np.zeros((1, C), np.float32)])
    dl = np.concatenate([local_idx[dst].astype(np.float32), [PAD_DST]])

    emb_p = ee[slot_edge]                    # [NBINS, S, 64]
    sh_p = es[slot_edge]                     # [NBINS, S, 16]
    x_p = xs[slot_edge]                      # [NBINS, S, 128]
    dl_p = dl[slot_edge]                     # [NBINS, S]
    dl_p[slot_edge < 0] = PAD_DST

    embt = np.ascontiguousarray(
        emb_p.reshape(N_CORES, CHUNKS, S, MLP_IN).transpose(0, 1, 3, 2))
    shp = np.ascontiguousarray(
        sh_p.reshape(N_CORES, CHUNKS, blocks, P, SH_DIM))
    xp = np.ascontiguousarray(
        x_p.reshape(N_CORES, CHUNKS, blocks, P, C))
    dstf = np.ascontiguousarray(
        dl_p.reshape(N_CORES, CHUNKS, blocks, P).transpose(0, 1, 3, 2))

    w1s = (W1 / np.sqrt(MLP_IN)).astype(np.float32)
    w2s = (W2 / np.sqrt(MLP_HID)).astype(np.float32)
    iota = np.tile(np.arange(P, dtype=np.float32), (P, 1))

    in_maps = []
    for c in range(N_CORES):
        in_maps.append({
            "embt": embt[c], "shp": shp[c], "xp": xp[c], "dstf": dstf[c],
            "w1s": w1s, "w2s": w2s, "iota": iota,
        })
    return in_maps, blocks, node_slot


_PERM = None


def _col_perm():
    """perm[ref_col] = dev_col ; ref col = 128*off_l + c*d_l + m,
    dev col = (off_l + m)*128 + c."""
    global _PERM
    if _PERM is None:
        perm = np.empty(OUT_COLS, np.int64)
        cs = np.arange(C)
        for li in range(4):
            d, off = D_L[li], OFF_L[li]
            for m in range(d):
                perm[C * off + cs * d + m] = (off + m) * C + cs
        _PERM = perm
    return _PERM


# ---------------------------------------------------------------------------
# entry points
# ---------------------------------------------------------------------------

def _run(inputs, trace=False, tmpdir=None):
    src_features = np.asarray(inputs["src_features"], np.float32)
    edge_sh = np.asarray(inputs["edge_sh"], np.float32)
    edge_emb = np.asarray(inputs["edge_emb"], np.float32)
    W1 = np.asarray(inputs["W1"], np.float32)
    W2 = np.asarray(inputs["W2"], np.float32)
    src = np.asarray(inputs["src"]).astype(np.int64)
    dst = np.asarray(inputs["dst"]).astype(np.int64)
    n_dst = int(inputs.get("num_dst_nodes", N_CORES * CHUNKS * P))

    in_maps, blocks, node_slot = _pack(
        src_features, edge_sh, edge_emb, W1, W2, src, dst, n_dst)
    nc = _get_nc(blocks)

    kw = {}
    if trace:
        from concourse import bass_utils as _bu  # noqa: F401
        _install_ntff_shim()
        kw = dict(trace=True, tmpdir=tmpdir)
    res = run_bass_kernel_spmd(nc, in_maps, core_ids=list(range(N_CORES)), **kw)

    dev = np.stack([res.results[c]["out"] for c in range(N_CORES)])
    dev_flat = dev.reshape(NBINS * P, OUT_COLS)
    out = dev_flat[node_slot][:, _col_perm()]
    return np.ascontiguousarray(out), res.exec_time_ns


def _install_ntff_shim():
    import types
    if "antenv.axon_hooks" in sys.modules:
        return
    mod = types.ModuleType("antenv.axon_hooks")
    mod._hook = None
    def set_axon_ntff_profile_hook(h):
        mod._hook = h
    def get_axon_ntff_profile_hook():
        return mod._hook
    mod.set_axon_ntff_profile_hook = set_axon_ntff_profile_hook
    mod.get_axon_ntff_profile_hook = get_axon_ntff_profile_hook
    sys.modules["antenv.axon_hooks"] = mod
    try:
        import antenv
        antenv.axon_hooks = mod
    except ImportError:
        pass
    try:
        from trn_agent_boot.trn_boot import _ntff_profile_via_ctypes
        set_axon_ntff_profile_hook(
            _ntff_profile_via_ctypes("/opt/axon/libaxon_pjrt.so"))
    except Exception:
        pass


def kernel(**inputs):
    out, _ = _run(inputs, trace=False)
    return out


# revision 17
# speedup vs baseline: 1.0067x; 1.0067x over previous
"""Trainium2 Bass kernel: MACE TensorProductConv (scalar-in x SH tensor product
+ per-edge MLP weights + segment_sum onto destination nodes).

Strategy (dst-sharded, no collectives):
  - Host bin-packs the N_DST destination nodes into 80 bins (<=128 nodes each,
    edge-balanced). Core k owns 10 bins ("chunks"); each chunk's node outputs
    accumulate in PSUM [128 nodes, 2048].
  - Edges are grouped by bin and padded to BLOCKS*128 slots per bin. Per
    128-edge block the device computes the edge MLP (2 matmuls + silu), the
    per-edge tensor product (DVE broadcast multiplies), and scatter-adds via a
    one-hot matmul: PSUM += onehot(dst_local)^T @ out_e.
  - Host pre-gathers src_features[src], pre-transposes edge_emb, pre-scales
    the MLP weights, and un-permutes the device output at the end.
"""

import sys
from contextlib import ExitStack

for _p in ("/opt/trn_rl_repo",):
    if _p not in sys.path:
        sys.path.insert(0, _p)

import numpy as np

import concourse.bacc as bacc
import concourse.tile as tile
import concourse.mybir as mybir
from concourse.bass_utils import run_bass_kernel_spmd

F32 = mybir.dt.float32
F32R = mybir.dt.float32r
BF16 = mybir.dt.bfloat16

N_CORES = 8
P = 128                 # partitions = edges per block = nodes per chunk
CHUNKS = 10             # bins per core
NBINS = N_CORES * CHUNKS
C = 128                 # channels
SH_DIM = 16
MLP_IN = 64
MLP_HID = 128
W_NUMEL = 512
OUT_COLS = 16 * C       # 2048
D_L = (1, 3, 5, 7)
OFF_L = (0, 1, 4, 9)    # m-offset of each l within the 16 sh columns
PAD_DST = 1000.0        # dst_local for padding slots -> one-hot row is all zero


# ---------------------------------------------------------------------------
# device program
# ---------------------------------------------------------------------------

_NC_CACHE = {}


def _build(blocks):
    nc = bacc.Bacc("TRN2", target_bir_lowering=False, debug=False,
                   num_devices=N_CORES)
    S = blocks * P
    embt = nc.dram_tensor("embt", [CHUNKS, MLP_IN, S], F32, kind="ExternalInput").ap()
    shp = nc.dram_tensor("shp", [CHUNKS, blocks, P, SH_DIM], F32, kind="ExternalInput").ap()
    xp = nc.dram_tensor("xp", [CHUNKS, blocks, P, C], F32, kind="ExternalInput").ap()
    dstf = nc.dram_tensor("dstf", [CHUNKS, P, blocks], F32, kind="ExternalInput").ap()
    w1s = nc.dram_tensor("w1s", [MLP_IN, MLP_HID], F32, kind="ExternalInput").ap()
    w2s = nc.dram_tensor("w2s", [MLP_HID, W_NUMEL], F32, kind="ExternalInput").ap()
    iota = nc.dram_tensor("iota", [P, P], F32, kind="ExternalInput").ap()
    outp = nc.dram_tensor("out", [CHUNKS, P, OUT_COLS], F32, kind="ExternalOutput").ap()

    with tile.TileContext(nc) as tc, ExitStack() as ctx:
        const = ctx.enter_context(tc.tile_pool(name="const", bufs=1))
        cpool = ctx.enter_context(tc.tile_pool(name="chunk", bufs=2))
        bpool = ctx.enter_context(tc.tile_pool(name="blk", bufs=3))
        opool = ctx.enter_context(tc.tile_pool(name="outs", bufs=2))
        ph_pool = ctx.enter_context(tc.tile_pool(name="ph", bufs=2, space="PSUM"))
        pw_pool = ctx.enter_context(tc.tile_pool(name="pw", bufs=2, space="PSUM"))
        pacc_pool = ctx.enter_context(tc.tile_pool(name="pacc", bufs=1, space="PSUM"))

        w1t = const.tile([MLP_IN, MLP_HID], F32)
        nc.sync.dma_start(w1t[:], w1s)
        w2t = const.tile([MLP_HID, W_NUMEL], F32)
        nc.sync.dma_start(w2t[:], w2s)
        iot = const.tile([P, P], F32)
        nc.sync.dma_start(iot[:], iota)

        for ch in range(CHUNKS):
            et = cpool.tile([MLP_IN, S], F32, tag="et")
            nc.sync.dma_start(et[:], embt[ch])
            xt = cpool.tile([P, S], F32, tag="xt")
            nc.sync.dma_start(xt[:].rearrange("p (b c) -> p b c", b=blocks),
                              xp[ch].rearrange("b i c -> i b c"))
            sht = cpool.tile([P, blocks * SH_DIM], F32, tag="sht")
            nc.sync.dma_start(sht[:].rearrange("p (b k) -> p b k", b=blocks),
                              shp[ch].rearrange("b i k -> i b k"))
            dtt = cpool.tile([P, blocks], F32, tag="dtt")
            nc.sync.dma_start(dtt[:], dstf[ch])

            acc = pacc_pool.tile([P, OUT_COLS], F32)

            for blk in range(blocks):
                # --- edge MLP: hT = silu(W1s^T @ emb^T), w = (hT)^T @ W2s ---
                # float32r: fp32 data, 4x faster PE streaming at N>=256
                ph = ph_pool.tile([MLP_HID, P], F32)
                nc.tensor.matmul(ph[:], lhsT=w1t[:],
                                 rhs=et[:, blk * P:(blk + 1) * P],
                                 start=True, stop=True)
                hs = bpool.tile([MLP_HID, P], F32, tag="hs")
                nc.scalar.activation(hs[:], ph[:], mybir.ActivationFunctionType.Silu)
                pw = pw_pool.tile([P, W_NUMEL], F32)
                nc.tensor.matmul(pw[:], lhsT=hs[:], rhs=w2t[:],
                                 start=True, stop=True)

                # --- A[e, l*C+c] = w[e, l*C+c] * x[e, c] ---
                xb = xt[:, blk * P:(blk + 1) * P]
                at = bpool.tile([P, W_NUMEL], F32, tag="at")
                nc.vector.tensor_mul(
                    at[:].rearrange("p (l c) -> p l c", l=4),
                    pw[:].rearrange("p (l c) -> p l c", l=4),
                    xb.unsqueeze(1).broadcast_to([P, 4, C]),
                )

                # --- one-hot of dst_local (bf16: 0/1 exact) ---
                oh = bpool.tile([P, P], BF16, tag="oh")
                nc.vector.tensor_scalar(oh[:], iot[:], dtt[:, blk:blk + 1], None,
                                        mybir.AluOpType.is_equal)

                # --- out_e[e, (off+m)*C + c] = A[e, l*C+c] * sh[e, off+m] ---
                # split across engines: l=0,2 -> DVE, l=1 -> ACT (copy w/
                # per-partition scale), l=3 -> GPSIMD
                oe = bpool.tile([P, OUT_COLS], BF16, tag="oe")
                for li, eng in ((0, nc.vector), (2, nc.vector), (3, nc.gpsimd)):
                    d, off = D_L[li], OFF_L[li]
                    eng.tensor_mul(
                        oe[:, off * C:(off + d) * C].rearrange("p (d c) -> p d c", d=d),
                        at[:, li * C:(li + 1) * C].unsqueeze(1).broadcast_to([P, d, C]),
                        sht[:, blk * SH_DIM + off: blk * SH_DIM + off + d]
                            .unsqueeze(2).broadcast_to([P, d, C]),
                    )
                for m in range(D_L[1]):
                    off = OFF_L[1]
                    nc.scalar.activation(
                        oe[:, (off + m) * C:(off + m + 1) * C],
                        at[:, C:2 * C],
                        mybir.ActivationFunctionType.Copy,
                        scale=sht[:, blk * SH_DIM + off + m: blk * SH_DIM + off + m + 1],
                    )

                # --- scatter-add: acc += onehot^T @ out_e ---
                for j in range(4):
                    nc.tensor.matmul(acc[:, j * 512:(j + 1) * 512],
                                     lhsT=oh[:], rhs=oe[:, j * 512:(j + 1) * 512],
                                     start=(blk == 0), stop=(blk == blocks - 1))

            ot = opool.tile([P, OUT_COLS], F32, tag="ot")
            nc.scalar.copy(ot[:], acc[:])
            nc.sync.dma_start(outp[ch], ot[:])

    nc.compile()
    return nc


def _get_nc(blocks):
    if blocks not in _NC_CACHE:
        _NC_CACHE[blocks] = _build(blocks)
    return _NC_CACHE[blocks]


# ---------------------------------------------------------------------------
# host-side packing
# ---------------------------------------------------------------------------

def _bin_pack(dst, n_dst):
    """Assign each dst node to one of NBINS bins: <=P nodes per bin,
    edge counts balanced (greedy LPT)."""
    import heapq
    deg = np.bincount(dst, minlength=n_dst)
    order = np.argsort(-deg, kind="stable")
    heap = [(0, b) for b in range(NBINS)]
    heapq.heapify(heap)
    bin_of_node = np.empty(n_dst, np.int32)
    bin_nnodes = np.zeros(NBINS, np.int32)
    bin_nedges = np.zeros(NBINS, np.int64)
    for n in order:
        while True:
            e, b = heapq.heappop(heap)
            if bin_nnodes[b] < P:
                break
        bin_of_node[n] = b
        bin_nnodes[b] += 1
        bin_nedges[b] = e + deg[n]
        if bin_nnodes[b] < P:
            heapq.heappush(heap, (int(bin_nedges[b]), b))
    return bin_of_node, bin_nnodes, bin_nedges


def _pack(src_features, edge_sh, edge_emb, W1, W2, src, dst, n_dst):
    E = dst.shape[0]
    bin_of_node, bin_nnodes, bin_nedges = _bin_pack(dst, n_dst)
    blocks = max(1, int(-(-int(bin_nedges.max()) // P)))  # ceil
    S = blocks * P

    # local node index within bin
    norder = np.argsort(bin_of_node, kind="stable")
    nstarts = np.concatenate([[0], np.cumsum(bin_nnodes)[:-1]])
    local_idx = np.empty(n_dst, np.int64)
    local_idx[norder] = np.arange(n_dst) - nstarts[bin_of_node[norder]]
    node_slot = bin_of_node.astype(np.int64) * P + local_idx

    # group edges by bin -> slot table [NBINS, S] of edge ids (-1 = padding)
    ebin = bin_of_node[dst]
    eorder = np.argsort(ebin, kind="stable")
    counts = np.bincount(ebin, minlength=NBINS)
    estarts = np.concatenate([[0], np.cumsum(counts)[:-1]])
    pos_sorted = np.arange(E) - estarts[ebin[eorder]]
    slot_edge = np.full((NBINS, S), -1, np.int64)
    slot_edge[ebin[eorder], pos_sorted] = eorder

    # gather per-slot data (index -1 -> appended zero row)
    ee = np.concatenate([edge_emb, np.zeros((1, MLP_IN), np.float32)])
    es = np.concatenate([edge_sh, np.zeros((1, SH_DIM), np.float32)])
    xs = np.concatenate([src_features[src], np.zeros((1, C), np.float32)])
    dl = np.concatenate([local_idx[dst].astype(np.float32), [PAD_DST]])

    emb_p = ee[slot_edge]                    # [NBINS, S, 64]
    sh_p = es[slot_edge]                     # [NBINS, S, 16]
    x_p = xs[slot_edge]                      # [NBINS, S, 128]
    dl_p = dl[slot_edge]                     # [NBINS, S]
    dl_p[slot_edge < 0] = PAD_DST

    embt = np.ascontiguousarray(
        emb_p.reshape(N_CORES, CHUNKS, S, MLP_IN).transpose(0, 1, 3, 2))
    shp = np.ascontiguousarray(
        sh_p.reshape(N_CORES, CHUNKS, blocks, P, SH_DIM))
    xp = np.ascontiguousarray(
        x_p.reshape(N_CORES, CHUNKS, blocks, P, C))
    dstf = np.ascontiguousarray(
        dl_p.reshape(N_CORES, CHUNKS, blocks, P).transpose(0, 1, 3, 2))

    w1s = (W1 / np.sqrt(MLP_IN)).astype(np.float32)
    w2s = (W2 / np.sqrt(MLP_HID)).astype(np.float32)
    iota = np.tile(np.arange(P, dtype=np.float32), (P, 1))

    in_maps = []
    for c in range(N_CORES):
        in_maps.append({
            "embt": embt[c], "shp": shp[c], "xp": xp[c], "dstf": dstf[c],
            "w1s": w1s, "w2s": w2s, "iota": iota,
        })
    return in_maps, blocks, node_slot


_PERM = None


def _col_perm():
    """perm[ref_col] = dev_col ; ref col = 128*off_l + c*d_l + m,
    dev col = (off_l + m)*128 + c."""
    global _PERM
    if _PERM is None:
        perm = np.empty(OUT_COLS, np.int64)
        cs = np.arange(C)
        for li in range(4):
            d, off = D_L[li], OFF_L[li]
            for m in range(d):
                perm[C * off + cs * d + m] = (off + m) * C + cs
        _PERM = perm
    return _PERM


# ---------------------------------------------------------------------------
# entry points
# ---------------------------------------------------------------------------

def _run(inputs, trace=False, tmpdir=None):
    src_features = np.asarray(inputs["src_features"], np.float32)
    edge_sh = np.asarray(inputs["edge_sh"], np.float32)
    edge_emb = np.asarray(inputs["edge_emb"], np.float32)
    W1 = np.asarray(inputs["W1"], np.float32)
    W2 = np.asarray(inputs["W2"], np.float32)
    src = np.asarray(inputs["src"]).astype(np.int64)
    dst = np.asarray(inputs["dst"]).astype(np.int64)
    n_dst = int(inputs.get("num_dst_nodes", N_CORES * CHUNKS * P))

    in_maps, blocks, node_slot = _pack(
        src_features, edge_sh, edge_emb, W1, W2, src, dst, n_dst)
    nc = _get_nc(blocks)

    kw = {}
    if trace:
        from concourse import bass_utils as _bu  # noqa: F401
        _install_ntff_shim()
        kw = dict(trace=True, tmpdir=tmpdir)
    res = run_bass_kernel_spmd(nc, in_maps, core_ids=list(range(N_CORES)), **kw)

    dev = np.stack([res.results[c]["out"] for c in range(N_CORES)])
    dev_flat = dev.reshape(NBINS * P, OUT_COLS)
    out = dev_flat[node_slot][:, _col_perm()]
    return np.ascontiguousarray(out), res.exec_time_ns


def _install_ntff_shim():
    import types
    if "antenv.axon_hooks" in sys.modules:
        return
    mod = types.ModuleType("antenv.axon_hooks")
    mod._hook = None
    def set_axon_ntff_profile_hook(h):
        mod._hook = h
    def get_axon_ntff_profile_hook():
        return mod._hook
    mod.set_axon_ntff_profile_hook = set_axon_ntff_profile_hook
    mod.get_axon_ntff_profile_hook = get_axon_ntff_profile_hook
    sys.modules["antenv.axon_hooks"] = mod
    try:
        import antenv
        antenv.axon_hooks = mod
    except ImportError:
        pass
    try:
        from trn_agent_boot.trn_boot import _ntff_profile_via_ctypes
        set_axon_ntff_profile_hook(
            _ntff_profile_via_ctypes("/opt/axon/libaxon_pjrt.so"))
    except Exception:
        pass


def kernel(**inputs):
    out, _ = _run(inputs, trace=False)
    return out
